# revision 18
# baseline (speedup 1.0000x reference)
"""Trainium2 Bass kernel for nn_CascadedAttention_76836964925817.

Math: the reference module's attention machinery is dead code — softmax over a
size-1 axis is identically 1, so `context = x[0].sum(axis=0)` is a constant
and the layer reduces to the 28-dim nonlinear recurrence

    y[t] = sigmoid(Wo @ y[t-1] + Uo @ x[t-1] + c),   c = Co @ sum_t x[t],
    y[-1] = 0, x[-1] := 0.

Strategy (collective-free; every core computes the full answer redundantly —
an AllGather-based variant spent ~55us of a ~100us kernel inside the
collective waiting on peer launch skew):
  * Each core streams the FULL x (8MB, fp32) from HBM as 8 sub-slab DMAs
    round-robined over both HWDGE rings (sync + scalar issue queues), with a
    slab-major SBUF layout so every DMA lands contiguously per partition.
  * U = Uo @ x.T (32 x 2048) is accumulated in PSUM with fp32r matmuls
    (1 cycle/column), one 512-column PSUM bank per T-slab, 8 contraction
    chunks each.  As each bank completes, its u rows are copied into the
    column-shifted SBUF tile usb (scalar ACT-copy), overlapped with the
    remaining x stream.  fp32r's ~12-bit mantissa costs only ~3e-4 on
    individual u values — harmless through the sigmoid.
  * c = Co @ sum_t x[t] must be much more precise than fp32r (the 2048-sum
    amplifies matmul rounding into a bias error), so the column sums s are
    computed EXACTLY with vector-engine reduces over the streamed x (hidden
    under the DMA), then c = Co @ s via eight tiny full-fp32 matmuls, then
    replicated across the 4 partition groups with one placement matmul and
    copied to SBUF as the activation bias.
  * Recurrence solved by Jacobi fixed-point sweeps (the map is a strong
    contraction: |sigmoid'| <= 1/4, ||Wo|| ~ 0.5; 3 sweeps reach ~1e-4 in
    both norm and max element error).  t is split into 4 column groups of
    512 stacked on partition blocks 32g..32g+27 (28 used + 4 pad).  Sweep
    k's PSUM bank is pre-filled with the B term (one eye matmul off the
    regrouped bg tile, overlapping sweep k-1's activation), then
        psum += blockdiag(Wo.T) @ YA[:, 0:512]    (shifted-y storage)
        psum += shiftblk(Wo.T) @ YA[:, 512:514]   (group boundary)
    and one 128-lane sigmoid ACT with per-partition bias c writes the next
    YA; the final sweep writes the output tile in two halves so the first
    output DMA overlaps the second sigmoid.

The kernel is self-contained: shapes/sharding are hardcoded.
"""

import numpy as np

import concourse.bass as bass
import concourse.mybir as mybir
import concourse.tile as tile
from concourse import bacc
from concourse import bass_utils

F32 = mybir.dt.float32
F32R = mybir.dt.float32r
BF16 = mybir.dt.bfloat16
AF = mybir.ActivationFunctionType

T, D, V = 2048, 1024, 28
N_CORES = 8
G = 4                      # column groups in the iteration phase
S = T // G                 # 512 columns per group
PB = 32                    # partition block stride per group (28 used + 4 pad)
PP = G * PB                # 128 partitions in the iteration phase
DCH = D // 128             # 8 contraction chunks
W2 = 64                    # padded [Uo;Co] rows: Uo 0:28, Co 32:60
K_SWEEPS = 3               # total Jacobi sweeps (incl. the B-only init sweep)


def build_body(nc, xt, w2t, wmm, eyep, crep, yg, tc=None, reps=1):
    """Emit the program. xt:(G,128,DCH,S) x slab-major fp32; w2t:(128,DCH*W2)
    zero-padded Uo.T; co_t:(128,DCH*V) Co.T chunks; wmm:(PP,2,PP)
    ([.,0,.]=blockdiag(Wo.T), [.,1,.]=boundary-shift(Wo.T)); eyep:(PP,PP)
    identity; crep:(V,PP) c-replication placement; yg:(PP,S) grouped out."""
    t = tc
    from contextlib import ExitStack
    ctx = ExitStack()
    sbp = ctx.enter_context(t.tile_pool(name="sb", bufs=1))
    pp = ctx.enter_context(t.tile_pool(name="pp", bufs=1, space="PSUM"))

    def st(shape, name, dt=F32):
        return sbp.tile(shape, dt, name=name, tag=name)

    xt_sb = st([128, G, 2, DCH, S], "xt_sb", BF16)
    w2t_sb = st([128, 2, DCH, W2], "w2t_sb", BF16)
    wmm_sb = st([PP, 2, PP], "wmm_sb", F32R)
    eyep_sb = st([PP, PP], "eyep_sb", F32R)
    crep_sb = st([W2, PP], "crep_sb", BF16)
    cfh = st([W2, 2], "cfh", BF16)
    cfl = st([W2, 2], "cfl", BF16)
    bg = st([PP, S], "bg", F32R)
    usb = st([V, T + 1], "usb", F32R)
    ya = st([PP, S + 2], "ya", F32R)
    yfin = st([PP, S], "yfin")
    cpart = st([W2, G + 1], "cpart")
    cfin = st([W2, 2], "cfin")
    cbias = st([PP, 1], "cbias")
    dummy = st([1, 1], "dummy")

    upsum = pp.tile([W2, T], F32, name="upsum", tag="upsum")
    ps = [pp.tile([PP, S], F32, name=f"ps{k}", tag=f"ps{k}")
          for k in range(K_SWEEPS)]
    cb_ps = pp.tile([PP, 2], F32, name="cb_ps", tag="cb_ps")

    # Early dummy sigmoid so the ACT table load happens off the critical path.
    nc.vector.memset(dummy[:, :], 0.0)
    nc.scalar.activation(out=dummy[:, :], in_=dummy[:, :], func=AF.Sigmoid)

    # one-time constants; w2t gates the first matmuls so it rides a fast
    # HWDGE ring, the rest take the SWDGE ring
    nc.scalar.dma_start(w2t_sb[:, :, :, :],
                        w2t.rearrange("p (h c v) -> p h c v", h=2, c=DCH))
    nc.gpsimd.dma_start(wmm_sb[:, :, :], wmm)
    nc.gpsimd.dma_start(eyep_sb[:, :], eyep)
    nc.gpsimd.dma_start(crep_sb[:, :], crep)
    nc.vector.memset(ya[:, :].bitcast(F32), 0.0)
    nc.vector.memset(usb[:, 0:1].bitcast(F32), 0.0)
    nc.vector.memset(bg[:, :].bitcast(F32), 0.0)
    nc.vector.memset(cfin[:, :], 0.0)
    nc.vector.memset(cfh[:, :].bitcast(mybir.dt.uint16), 0)
    nc.vector.memset(cfl[:, :].bitcast(mybir.dt.uint16), 0)

    prev_last = None
    for _rep in range(reps):
        prev_last = emit_rep(nc, t, xt, yg,
                             xt_sb, w2t_sb, wmm_sb, eyep_sb,
                             crep_sb, bg, usb, ya, yfin, cpart, cfin,
                             cfh, cfl, cbias, upsum, ps, cb_ps, prev_last)
    ctx.close()


def emit_rep(nc, t, xt, yg, xt_sb, w2t_sb, wmm_sb, eyep_sb,
             crep_sb, bg, usb, ya, yfin, cpart, cfin, cfh, cfl, cbias,
             upsum, ps, cb_ps, prev_last=None):
    from concourse.tile_rust import add_dep_helper

    # ------- stream x: 8 sub-slab DMAs round-robined over both rings -------
    rings = [nc.sync, nc.scalar, nc.gpsimd]
    for q in range(G):
        for h in range(2):
            eng = rings[(2 * q + h) % 3]
            d = eng.dma_start(xt_sb[:, q, h, :, :], xt[q, :, h, :, :])
            if q == 0 and h == 0 and prev_last is not None:
                add_dep_helper(d.ins, prev_last.ins,
                               reason="serialize reps for latency measurement")

    # -------- U = Uo @ x.T -> (32, 2048) fp32r, one bank per slab ----------
    # As each bank finishes, its u rows are copied into the shifted usb
    # window (scalar ACT-copy) and regrouped into bg (SBUF->SBUF DMA).
    # In parallel the vector engine accumulates exact per-chunk column sums
    # of x for the c path.
    terms = [(0, 0), (1, 0), (0, 1)]   # (w half, x half); lo*lo dropped;
    nmm = DCH * len(terms)             # hi-x terms first (lo DMA lands later)
    for q in range(G):
        # the last slab accumulates in column halves so its copy/regroup
        # pipeline starts half a bank early
        halves = [(0, S)] if q < G - 1 else [(0, S // 2), (S // 2, S)]
        for lo, hi in halves:
            i = 0
            for hw, hx in terms:
                for c in range(DCH):
                    i += 1
                    nc.tensor.matmul(
                        upsum[:, S * q + lo:S * q + hi],
                        lhsT=w2t_sb[:, hw, c, :],
                        rhs=xt_sb[:, q, hx, c, lo:hi],
                        start=(i == 1),
                        stop=(i == nmm),
                    )
            nc.vector.tensor_reduce(
                out=cpart[32:32 + V, q + (1 if lo else 0):
                          q + (2 if lo else 1)],
                in_=upsum[32:32 + V, S * q + lo:S * q + hi],
                axis=mybir.AxisListType.X, op=mybir.AluOpType.add,
            )
            nc.scalar.copy(usb[:, 1 + S * q + lo:1 + S * q + hi],
                           upsum[0:V, S * q + lo:S * q + hi])
            deng = nc.sync if (q + (0 if lo else 1)) % 2 == 0 else nc.scalar
            deng.dma_start(bg[PB * q:PB * q + V, lo:hi],
                           usb[:, S * q + lo:S * q + hi])

    # ------- c path: total the per-bank Co-row partials, replicate --------
    # cfin is split hi/lo into bf16 halves so the replication matmul's
    # moving operand loses nothing (the PE truncates moving fp32 data).
    nc.vector.tensor_reduce(out=cfin[32:32 + V, 0:1],
                            in_=cpart[32:32 + V, :],
                            axis=mybir.AxisListType.X, op=mybir.AluOpType.add)
    nc.vector.tensor_copy(cfh[32:32 + V, 0:1], cfin[32:32 + V, 0:1])
    nc.vector.tensor_tensor(cfl[32:32 + V, 0:1],
                            cfin[32:32 + V, 0:1], cfh[32:32 + V, 0:1],
                            mybir.AluOpType.subtract)
    nc.tensor.matmul(cb_ps[:, :], lhsT=crep_sb[32:32 + V, :],
                     rhs=cfh[32:32 + V, :], start=True, stop=False)
    nc.tensor.matmul(cb_ps[:, :], lhsT=crep_sb[32:32 + V, :],
                     rhs=cfl[32:32 + V, :], start=False, stop=True)
    nc.vector.tensor_copy(cbias[:, :], cb_ps[:, 0:1])

    # ---------------- Jacobi sweeps ----------------
    # YA[32g+v, j] stores y[512g + j - 1] for j in 1..512; col 0 and col 513
    # are permanent zeros.  bg[32g+v, tau] = u[512g + tau - 1] (regrouped via
    # DMA; pad rows stay zero).  All prefills are emitted first so the PE
    # works through them while activations run.
    for k in range(K_SWEEPS):
        nc.tensor.matmul(ps[k][:, :], lhsT=eyep_sb[:, :], rhs=bg[:, :],
                         start=True, stop=(k == 0))
    for k in range(K_SWEEPS):
        if k > 0:
            nc.tensor.matmul(ps[k][:, :], lhsT=wmm_sb[:, 0, :],
                             rhs=ya[:, 0:S], start=False, stop=False)
            nc.tensor.matmul(ps[k][:, 0:2], lhsT=wmm_sb[:, 1, :],
                             rhs=ya[:, S:S + 2], start=False, stop=True)
        if k < K_SWEEPS - 1:
            nc.scalar.activation(out=ya[:, 1:S + 1], in_=ps[k][:, :],
                                 func=AF.Sigmoid, bias=cbias[:, 0:1],
                                 scale=1.0)
        else:
            # halves, so the first output DMA overlaps the second sigmoid
            nc.scalar.activation(out=yfin[:, 0:S // 2],
                                 in_=ps[k][:, 0:S // 2],
                                 func=AF.Sigmoid, bias=cbias[:, 0:1],
                                 scale=1.0)
            nc.scalar.activation(out=yfin[:, S // 2:S],
                                 in_=ps[k][:, S // 2:S],
                                 func=AF.Sigmoid, bias=cbias[:, 0:1],
                                 scale=1.0)

    # ---------------- write grouped output ----------------
    nc.sync.dma_start(yg[:, 0:S // 2], yfin[:, 0:S // 2])
    return nc.scalar.dma_start(yg[:, S // 2:S], yfin[:, S // 2:S])


_CACHED_NC = {}


def _get_nc(reps=1):
    if reps not in _CACHED_NC:
        nc = bacc.Bacc("TRN2", target_bir_lowering=False, debug=False,
                       num_devices=N_CORES)
        xt = nc.dram_tensor("xt", [G, 128, 2, DCH, S], BF16,
                            kind="ExternalInput")
        w2t = nc.dram_tensor("w2t", [128, 2 * DCH * W2], BF16,
                             kind="ExternalInput")
        wmm = nc.dram_tensor("wmm", [PP, 2, PP], F32R, kind="ExternalInput")
        eyep = nc.dram_tensor("eyep", [PP, PP], F32R, kind="ExternalInput")
        crep = nc.dram_tensor("crep", [W2, PP], BF16, kind="ExternalInput")
        yg = nc.dram_tensor("yg", [PP, S], F32, kind="ExternalOutput")
        with tile.TileContext(nc) as t:
            build_body(nc, xt.ap(), w2t.ap(), wmm.ap(),
                       eyep.ap(), crep.ap(), yg.ap(), tc=t, reps=reps)
        nc.compile()
        _CACHED_NC[reps] = nc
    return _CACHED_NC[reps]


def _hilo(a):
    """Split fp32 array into (hi, lo) bf16 parts: a ~ hi + lo."""
    import ml_dtypes
    hi = a.astype(ml_dtypes.bfloat16)
    lo = (a - hi.astype(np.float32)).astype(ml_dtypes.bfloat16)
    return hi, lo


def make_in_maps(x, Uo, Co, Wo):
    import ml_dtypes
    xb = np.ascontiguousarray(np.asarray(x, np.float32)[0])        # (T, D)
    # xt[q, p, h, c, tau] = hilo(x[512q + tau, 128c + p])[h]
    xf = np.ascontiguousarray(
        xb.T.reshape(DCH, 128, G, S).transpose(2, 1, 0, 3))
    xh, xl = _hilo(xf)
    xt = np.ascontiguousarray(np.stack([xh, xl], axis=2))
    w2 = np.zeros((W2, D), np.float32)
    w2[0:V] = np.asarray(Uo, np.float32)
    w2[32:32 + V] = np.asarray(Co, np.float32)
    # w2t[p, (h, c, j)] = hilo(w2[j, 128c + p])[h]
    w2f = np.ascontiguousarray(
        w2.T.reshape(DCH, 128, W2).transpose(1, 0, 2))
    wh, wl = _hilo(w2f)
    w2t = np.ascontiguousarray(
        np.stack([wh, wl], axis=1).reshape(128, 2 * DCH * W2))
    wot = np.ascontiguousarray(np.asarray(Wo, np.float32).T)       # (V, V)
    wmm = np.zeros((PP, 2, PP), np.float32)
    for g in range(G):
        wmm[PB * g:PB * g + V, 0, PB * g:PB * g + V] = wot
        if g > 0:
            wmm[PB * (g - 1):PB * (g - 1) + V, 1, PB * g:PB * g + V] = wot
    eyep = np.eye(PP, dtype=np.float32)
    crep = np.zeros((W2, PP), ml_dtypes.bfloat16)
    for g in range(G):
        crep[32:32 + V, PB * g:PB * g + V] = np.eye(V, dtype=np.float32)
    in_map = {"xt": xt, "w2t": w2t, "wmm": wmm, "eyep": eyep, "crep": crep}
    return [in_map for _ in range(N_CORES)]


def unshard_output(yg):
    y = np.empty((T, V), np.float32)
    for g in range(G):
        y[g * S:(g + 1) * S, :] = yg[PB * g:PB * g + V, :].T
    return y[None]


def run(inputs, trace=False, reps=1, **kw):
    nc = _get_nc(reps)
    in_maps = make_in_maps(inputs["x"], inputs["Uo"], inputs["Co"],
                           inputs["Wo"])
    res = bass_utils.run_bass_kernel_spmd(
        nc, in_maps, core_ids=list(range(N_CORES)), trace=trace, **kw)
    return unshard_output(res.results[0]["yg"]), res


def kernel(**inputs):
    out, _ = run(inputs)
    return out


# revision 19
# speedup vs baseline: 1.2218x; 1.2218x over previous
"""Trainium2 Bass kernel for nn_CascadedAttention_76836964925817.

Math: the reference module's attention machinery is dead code — softmax over a
size-1 axis is identically 1, so `context = x[0].sum(axis=0)` is a constant
and the layer reduces to the 28-dim nonlinear recurrence

    y[t] = sigmoid(Wo @ y[t-1] + Uo @ x[t-1] + c),   c = Co @ sum_t x[t],
    y[-1] = 0, x[-1] := 0.

Strategy (collective-free; every core computes the full answer redundantly —
an AllGather-based variant spent ~55us of a ~100us kernel inside the
collective waiting on peer launch skew):
  * Each core streams the FULL x (8MB, fp32) from HBM as 8 sub-slab DMAs
    round-robined over both HWDGE rings (sync + scalar issue queues), with a
    slab-major SBUF layout so every DMA lands contiguously per partition.
  * U = Uo @ x.T (32 x 2048) is accumulated in PSUM with fp32r matmuls
    (1 cycle/column), one 512-column PSUM bank per T-slab, 8 contraction
    chunks each.  As each bank completes, its u rows are copied into the
    column-shifted SBUF tile usb (scalar ACT-copy), overlapped with the
    remaining x stream.  fp32r's ~12-bit mantissa costs only ~3e-4 on
    individual u values — harmless through the sigmoid.
  * c = Co @ sum_t x[t] must be much more precise than fp32r (the 2048-sum
    amplifies matmul rounding into a bias error), so the column sums s are
    computed EXACTLY with vector-engine reduces over the streamed x (hidden
    under the DMA), then c = Co @ s via eight tiny full-fp32 matmuls, then
    replicated across the 4 partition groups with one placement matmul and
    copied to SBUF as the activation bias.
  * Recurrence solved by Jacobi fixed-point sweeps (the map is a strong
    contraction: |sigmoid'| <= 1/4, ||Wo|| ~ 0.5; 3 sweeps reach ~1e-4 in
    both norm and max element error).  t is split into 4 column groups of
    512 stacked on partition blocks 32g..32g+27 (28 used + 4 pad).  Sweep
    k's PSUM bank is pre-filled with the B term (one eye matmul off the
    regrouped bg tile, overlapping sweep k-1's activation), then
        psum += blockdiag(Wo.T) @ YA[:, 0:512]    (shifted-y storage)
        psum += shiftblk(Wo.T) @ YA[:, 512:514]   (group boundary)
    and one 128-lane sigmoid ACT with per-partition bias c writes the next
    YA; the final sweep writes the output tile in two halves so the first
    output DMA overlaps the second sigmoid.

The kernel is self-contained: shapes/sharding are hardcoded.
"""

import numpy as np

import concourse.bass as bass
import concourse.mybir as mybir
import concourse.tile as tile
from concourse import bacc
from concourse import bass_utils

F32 = mybir.dt.float32
F32R = mybir.dt.float32r
BF16 = mybir.dt.bfloat16
AF = mybir.ActivationFunctionType

T, D, V = 2048, 1024, 28
N_CORES = 8
G = 4                      # column groups in the iteration phase
S = T // G                 # 512 columns per group
PB = 32                    # partition block stride per group (28 used + 4 pad)
PP = G * PB                # 128 partitions in the iteration phase
DCH = D // 128             # 8 contraction chunks
W2 = 64                    # padded [Uo;Co] rows: Uo 0:28, Co 32:60
K_SWEEPS = 3               # total Jacobi sweeps (incl. the B-only init sweep)


def build_body(nc, xt, w2t, wmm, eyep, crep, yg, tc=None, reps=1):
    """Emit the program. xt:(G,128,DCH,S) x slab-major fp32; w2t:(128,DCH*W2)
    zero-padded Uo.T; co_t:(128,DCH*V) Co.T chunks; wmm:(PP,2,PP)
    ([.,0,.]=blockdiag(Wo.T), [.,1,.]=boundary-shift(Wo.T)); eyep:(PP,PP)
    identity; crep:(V,PP) c-replication placement; yg:(PP,S) grouped out."""
    t = tc
    from contextlib import ExitStack
    ctx = ExitStack()
    sbp = ctx.enter_context(t.tile_pool(name="sb", bufs=1))
    pp = ctx.enter_context(t.tile_pool(name="pp", bufs=1, space="PSUM"))

    def st(shape, name, dt=F32):
        return sbp.tile(shape, dt, name=name, tag=name)

    xt_sb = st([128, G, 2, DCH, S], "xt_sb", BF16)
    w2t_sb = st([128, 2, DCH, W2], "w2t_sb", BF16)
    wmm_sb = st([PP, 2, PP], "wmm_sb", F32R)
    eyep_sb = st([PP, PP], "eyep_sb", F32R)
    crep_sb = st([W2, PP], "crep_sb", BF16)
    cfh = st([W2, 2], "cfh", BF16)
    cfl = st([W2, 2], "cfl", BF16)
    bg = st([PP, S], "bg", F32R)
    usb = st([V, T + 1], "usb", F32R)
    ya = st([PP, S + 2], "ya", F32R)
    yfin = st([PP, S], "yfin")
    cpart = st([W2, G + 1], "cpart")
    cfin = st([W2, 2], "cfin")
    cbias = st([PP, 1], "cbias")
    dummy = st([1, 1], "dummy")

    upsum = pp.tile([W2, T], F32, name="upsum", tag="upsum")
    ps = [pp.tile([PP, S], F32, name=f"ps{k}", tag=f"ps{k}")
          for k in range(K_SWEEPS)]
    cb_ps = pp.tile([PP, 2], F32, name="cb_ps", tag="cb_ps")

    # Early dummy sigmoid so the ACT table load happens off the critical path.
    nc.vector.memset(dummy[:, :], 0.0)
    nc.scalar.activation(out=dummy[:, :], in_=dummy[:, :], func=AF.Sigmoid)

    # one-time constants; w2t gates the first matmuls so it rides a fast
    # HWDGE ring, the rest take the SWDGE ring
    nc.scalar.dma_start(w2t_sb[:, :, :, :],
                        w2t.rearrange("p (h c v) -> p h c v", h=2, c=DCH))
    nc.gpsimd.dma_start(wmm_sb[:, :, :], wmm)
    nc.gpsimd.dma_start(eyep_sb[:, :], eyep)
    nc.gpsimd.dma_start(crep_sb[:, :], crep)
    nc.vector.memset(ya[:, :].bitcast(F32), 0.0)
    nc.vector.memset(usb[:, 0:1].bitcast(F32), 0.0)
    nc.vector.memset(bg[:, :].bitcast(F32), 0.0)
    nc.vector.memset(cfin[:, :], 0.0)
    nc.vector.memset(cfh[:, :].bitcast(mybir.dt.uint16), 0)
    nc.vector.memset(cfl[:, :].bitcast(mybir.dt.uint16), 0)

    prev_last = None
    for _rep in range(reps):
        prev_last = emit_rep(nc, t, xt, yg,
                             xt_sb, w2t_sb, wmm_sb, eyep_sb,
                             crep_sb, bg, usb, ya, yfin, cpart, cfin,
                             cfh, cfl, cbias, upsum, ps, cb_ps, prev_last)
    ctx.close()


def emit_rep(nc, t, xt, yg, xt_sb, w2t_sb, wmm_sb, eyep_sb,
             crep_sb, bg, usb, ya, yfin, cpart, cfin, cfh, cfl, cbias,
             upsum, ps, cb_ps, prev_last=None):
    from concourse.tile_rust import add_dep_helper

    # ------- stream x: 8 sub-slab DMAs round-robined over both rings -------
    for q in range(G):
        for h in range(2):
            eng = nc.sync if (2 * q + h) % 2 == 0 else nc.scalar
            d = eng.dma_start(xt_sb[:, q, h, :, :], xt[q, :, h, :, :])
            if q == 0 and h == 0 and prev_last is not None:
                add_dep_helper(d.ins, prev_last.ins,
                               reason="serialize reps for latency measurement")

    # -------- U = Uo @ x.T -> (32, 2048) fp32r, one bank per slab ----------
    # As each bank finishes, its u rows are copied into the shifted usb
    # window (scalar ACT-copy) and regrouped into bg (SBUF->SBUF DMA).
    # In parallel the vector engine accumulates exact per-chunk column sums
    # of x for the c path.
    terms = [(0, 0), (1, 0), (0, 1)]   # (w half, x half); lo*lo dropped;
    nmm = DCH * len(terms)             # hi-x terms first (lo DMA lands later)
    for q in range(G):
        # the last slab accumulates in column halves so its copy/regroup
        # pipeline starts half a bank early
        halves = [(0, S)] if q < G - 1 else [(0, S // 2), (S // 2, S)]
        for lo, hi in halves:
            i = 0
            for hw, hx in terms:
                for c in range(DCH):
                    i += 1
                    nc.tensor.matmul(
                        upsum[:, S * q + lo:S * q + hi],
                        lhsT=w2t_sb[:, hw, c, :],
                        rhs=xt_sb[:, q, hx, c, lo:hi],
                        start=(i == 1),
                        stop=(i == nmm),
                    )
            nc.vector.tensor_reduce(
                out=cpart[32:32 + V, q + (1 if lo else 0):
                          q + (2 if lo else 1)],
                in_=upsum[32:32 + V, S * q + lo:S * q + hi],
                axis=mybir.AxisListType.X, op=mybir.AluOpType.add,
            )
            nc.scalar.copy(usb[:, 1 + S * q + lo:1 + S * q + hi],
                           upsum[0:V, S * q + lo:S * q + hi])
            deng = nc.sync if (q + (0 if lo else 1)) % 2 == 0 else nc.scalar
            deng.dma_start(bg[PB * q:PB * q + V, lo:hi],
                           usb[:, S * q + lo:S * q + hi])

    # ------- c path: total the per-bank Co-row partials, replicate --------
    # cfin is split hi/lo into bf16 halves so the replication matmul's
    # moving operand loses nothing (the PE truncates moving fp32 data).
    nc.vector.tensor_reduce(out=cfin[32:32 + V, 0:1],
                            in_=cpart[32:32 + V, :],
                            axis=mybir.AxisListType.X, op=mybir.AluOpType.add)
    nc.vector.tensor_copy(cfh[32:32 + V, 0:1], cfin[32:32 + V, 0:1])
    nc.vector.tensor_tensor(cfl[32:32 + V, 0:1],
                            cfin[32:32 + V, 0:1], cfh[32:32 + V, 0:1],
                            mybir.AluOpType.subtract)
    nc.tensor.matmul(cb_ps[:, :], lhsT=crep_sb[32:32 + V, :],
                     rhs=cfh[32:32 + V, :], start=True, stop=False)
    nc.tensor.matmul(cb_ps[:, :], lhsT=crep_sb[32:32 + V, :],
                     rhs=cfl[32:32 + V, :], start=False, stop=True)
    nc.vector.tensor_copy(cbias[:, :], cb_ps[:, 0:1])

    # ---------------- Jacobi sweeps ----------------
    # YA[32g+v, j] stores y[512g + j - 1] for j in 1..512; col 0 and col 513
    # are permanent zeros.  bg[32g+v, tau] = u[512g + tau - 1] (regrouped via
    # DMA; pad rows stay zero).  All prefills are emitted first so the PE
    # works through them while activations run.
    for k in range(K_SWEEPS):
        nc.tensor.matmul(ps[k][:, :], lhsT=eyep_sb[:, :], rhs=bg[:, :],
                         start=True, stop=(k == 0))
    for k in range(K_SWEEPS):
        if k > 0:
            nc.tensor.matmul(ps[k][:, :], lhsT=wmm_sb[:, 0, :],
                             rhs=ya[:, 0:S], start=False, stop=False)
            nc.tensor.matmul(ps[k][:, 0:2], lhsT=wmm_sb[:, 1, :],
                             rhs=ya[:, S:S + 2], start=False, stop=True)
        if k < K_SWEEPS - 1:
            nc.scalar.activation(out=ya[:, 1:S + 1], in_=ps[k][:, :],
                                 func=AF.Sigmoid, bias=cbias[:, 0:1],
                                 scale=1.0)
        else:
            # halves, so the first output DMA overlaps the second sigmoid
            nc.scalar.activation(out=yfin[:, 0:S // 2],
                                 in_=ps[k][:, 0:S // 2],
                                 func=AF.Sigmoid, bias=cbias[:, 0:1],
                                 scale=1.0)
            nc.scalar.activation(out=yfin[:, S // 2:S],
                                 in_=ps[k][:, S // 2:S],
                                 func=AF.Sigmoid, bias=cbias[:, 0:1],
                                 scale=1.0)

    # ---------------- write grouped output ----------------
    nc.sync.dma_start(yg[:, 0:S // 2], yfin[:, 0:S // 2])
    return nc.scalar.dma_start(yg[:, S // 2:S], yfin[:, S // 2:S])


_CACHED_NC = {}


def _get_nc(reps=1):
    if reps not in _CACHED_NC:
        nc = bacc.Bacc("TRN2", target_bir_lowering=False, debug=False,
                       num_devices=N_CORES)
        xt = nc.dram_tensor("xt", [G, 128, 2, DCH, S], BF16,
                            kind="ExternalInput")
        w2t = nc.dram_tensor("w2t", [128, 2 * DCH * W2], BF16,
                             kind="ExternalInput")
        wmm = nc.dram_tensor("wmm", [PP, 2, PP], F32R, kind="ExternalInput")
        eyep = nc.dram_tensor("eyep", [PP, PP], F32R, kind="ExternalInput")
        crep = nc.dram_tensor("crep", [W2, PP], BF16, kind="ExternalInput")
        yg = nc.dram_tensor("yg", [PP, S], F32, kind="ExternalOutput")
        with tile.TileContext(nc) as t:
            build_body(nc, xt.ap(), w2t.ap(), wmm.ap(),
                       eyep.ap(), crep.ap(), yg.ap(), tc=t, reps=reps)
        nc.compile()
        _CACHED_NC[reps] = nc
    return _CACHED_NC[reps]


def _hilo(a):
    """Split fp32 array into (hi, lo) bf16 parts: a ~ hi + lo."""
    import ml_dtypes
    hi = a.astype(ml_dtypes.bfloat16)
    lo = (a - hi.astype(np.float32)).astype(ml_dtypes.bfloat16)
    return hi, lo


def make_in_maps(x, Uo, Co, Wo):
    import ml_dtypes
    xb = np.ascontiguousarray(np.asarray(x, np.float32)[0])        # (T, D)
    # xt[q, p, h, c, tau] = hilo(x[512q + tau, 128c + p])[h]
    xf = np.ascontiguousarray(
        xb.T.reshape(DCH, 128, G, S).transpose(2, 1, 0, 3))
    xh, xl = _hilo(xf)
    xt = np.ascontiguousarray(np.stack([xh, xl], axis=2))
    w2 = np.zeros((W2, D), np.float32)
    w2[0:V] = np.asarray(Uo, np.float32)
    w2[32:32 + V] = np.asarray(Co, np.float32)
    # w2t[p, (h, c, j)] = hilo(w2[j, 128c + p])[h]
    w2f = np.ascontiguousarray(
        w2.T.reshape(DCH, 128, W2).transpose(1, 0, 2))
    wh, wl = _hilo(w2f)
    w2t = np.ascontiguousarray(
        np.stack([wh, wl], axis=1).reshape(128, 2 * DCH * W2))
    wot = np.ascontiguousarray(np.asarray(Wo, np.float32).T)       # (V, V)
    wmm = np.zeros((PP, 2, PP), np.float32)
    for g in range(G):
        wmm[PB * g:PB * g + V, 0, PB * g:PB * g + V] = wot
        if g > 0:
            wmm[PB * (g - 1):PB * (g - 1) + V, 1, PB * g:PB * g + V] = wot
    eyep = np.eye(PP, dtype=np.float32)
    crep = np.zeros((W2, PP), ml_dtypes.bfloat16)
    for g in range(G):
        crep[32:32 + V, PB * g:PB * g + V] = np.eye(V, dtype=np.float32)
    in_map = {"xt": xt, "w2t": w2t, "wmm": wmm, "eyep": eyep, "crep": crep}
    return [in_map for _ in range(N_CORES)]


def unshard_output(yg):
    y = np.empty((T, V), np.float32)
    for g in range(G):
        y[g * S:(g + 1) * S, :] = yg[PB * g:PB * g + V, :].T
    return y[None]


def run(inputs, trace=False, reps=1, **kw):
    nc = _get_nc(reps)
    in_maps = make_in_maps(inputs["x"], inputs["Uo"], inputs["Co"],
                           inputs["Wo"])
    res = bass_utils.run_bass_kernel_spmd(
        nc, in_maps, core_ids=list(range(N_CORES)), trace=trace, **kw)
    return unshard_output(res.results[0]["yg"]), res


def kernel(**inputs):
    out, _ = run(inputs)
    return out
